# revision 5
# baseline (speedup 1.0000x reference)
"""Trainium2 Bass kernel for a spectral-normed linear + Ricker-wavelet KAN layer.

Math (per token row x_n in R^1024):
  base  = silu(x_n) @ (base_w/sigma).T + base_b
  xn    = tanh(x_n) * 2.5
  basis = (1 - xe^2) * exp(-xe^2/2),  xe = (xn - t_g)/s_g   (7 wavelets per feature)
  kan   = softshrink(basis_flat @ wavelet_w.T, thr=softplus(soft_threshold))
  out   = (base + kan) * output_scale

Strategy: data-parallel across 8 NeuronCores (2048 tokens each), weights
replicated.  Matmuls run in bf16 (fp32 PSUM accumulate).  Per 512-token
super-tile: base matmuls accumulate in PSUM then drain to SBUF, wavelet
matmuls reuse the freed PSUM banks, and the soft-threshold epilogue uses
relu(kan-thr) - relu(-kan-thr) so ScalarE can read PSUM directly.
The wavelet weights are negated on the host so the Ricker basis can be
computed as (xe^2 - 1)*exp(-xe^2/2) in a single fused DVE op.
"""

import sys

if '/opt/trn_rl_repo' not in sys.path:
    sys.path.insert(0, '/opt/trn_rl_repo')

import numpy as np
import ml_dtypes

import concourse.bass as bass
import concourse.mybir as mybir
import concourse.tile as tile
from concourse import bacc
from concourse.bass_utils import run_bass_kernel_spmd

N_CORES = 8
BATCH, SEQ, IN_F, OUT_F, GRID = 4, 4096, 1024, 1024, 7
NTOK = BATCH * SEQ            # 16384 tokens
TPC = NTOK // N_CORES         # 2048 tokens per core
ST = 512                      # tokens per super-tile
NST = TPC // ST               # 4 super-tiles per core
NIC = IN_F // 128             # 8 input-feature chunks
NTT = ST // 128               # 4 token tiles per super-tile
NH = OUT_F // 512             # 2 output halves

F32 = mybir.dt.float32
BF16 = mybir.dt.bfloat16
AF = mybir.ActivationFunctionType
OP = mybir.AluOpType

_BUILD_CACHE = {}


def _build_nc(a_g, b_g, thr, out_scale, has_bias):
    """Build + compile the SPMD Bass program.  a_g/b_g are the per-wavelet
    affine coefficients for the Square activation: xe^2 = (a*tanh(x)+b)^2."""
    nc = bacc.Bacc("TRN2", target_bir_lowering=False, debug=False,
                   num_devices=N_CORES)

    xT = nc.dram_tensor("xT", [IN_F, TPC], F32, kind="ExternalInput")
    ww = nc.dram_tensor("ww", [GRID * NIC, 128, OUT_F], BF16, kind="ExternalInput")
    wsn = nc.dram_tensor("wsn", [NIC, 128, OUT_F], BF16, kind="ExternalInput")
    bias = nc.dram_tensor("bias", [1, OUT_F], BF16, kind="ExternalInput")
    out = nc.dram_tensor("out", [TPC, OUT_F], F32, kind="ExternalOutput")

    with tile.TileContext(nc) as tc:
        with (
            tc.tile_pool(name="wpool", bufs=1) as wpool,
            tc.tile_pool(name="xpool", bufs=3) as xpool,
            tc.tile_pool(name="xnpool", bufs=NIC + 1) as xnpool,
            tc.tile_pool(name="tpool", bufs=3) as tpool,
            tc.tile_pool(name="bpool", bufs=12) as bpool,
            tc.tile_pool(name="epool", bufs=4) as epool,
            tc.tile_pool(name="opool", bufs=3) as opool,
            tc.tile_pool(name="psum", bufs=8, space="PSUM") as pp,
        ):
            # ---- constant bias columns for activation ops ----
            _consts = {}

            def const_col(val):
                val = float(val)
                if val not in _consts:
                    t = wpool.tile([128, 1], F32, name=f"const{len(_consts)}")
                    nc.gpsimd.memset(t[:], val)
                    _consts[val] = t
                return _consts[val][:]

            # ---- resident weights ----
            ww_sb = wpool.tile([128, GRID * NIC * OUT_F], BF16)
            for j in range(GRID * NIC):
                nc.sync.dma_start(
                    out=ww_sb[:, j * OUT_F:(j + 1) * OUT_F], in_=ww.ap()[j])
            wsn_sb = wpool.tile([128, NIC * OUT_F], BF16)
            for j in range(NIC):
                nc.sync.dma_start(
                    out=wsn_sb[:, j * OUT_F:(j + 1) * OUT_F], in_=wsn.ap()[j])
            if has_bias:
                bias_sb = wpool.tile([1, OUT_F], BF16)
                nc.sync.dma_start(out=bias_sb[:], in_=bias.ap())
                ones_sb = wpool.tile([1, 128], BF16)
                nc.vector.memset(ones_sb[:], 1.0)

            for st in range(NST):
                s0 = st * ST
                # ---- phase A: base = silu(x) @ w_sn.T (PSUM), tanh cache ----
                psum_b = [[pp.tile([128, 512], F32, tag="ps", name=f"psb_{st}_{tt}_{h}")
                           for h in range(NH)] for tt in range(NTT)]
                xn_tiles = []
                for ic in range(NIC):
                    x_t = xpool.tile([128, ST], F32, tag="x")
                    nc.sync.dma_start(
                        out=x_t[:], in_=xT.ap()[ic * 128:(ic + 1) * 128, s0:s0 + ST])
                    th2 = tpool.tile([128, ST], F32, tag="th2")
                    nc.scalar.activation(th2[:], x_t[:], AF.Tanh, scale=0.5)
                    silu2 = tpool.tile([128, ST], BF16, tag="silu2")
                    # 2*silu(x) = (tanh(x/2)+1)*x ; wsn carries the 1/2
                    nc.vector.scalar_tensor_tensor(
                        silu2[:], th2[:], 1.0, x_t[:], OP.add, OP.mult)
                    xn = xnpool.tile([128, ST], F32, tag="xn")
                    nc.scalar.activation(xn[:], x_t[:], AF.Tanh, scale=1.0)
                    xn_tiles.append(xn)
                    for tt in range(NTT):
                        for h in range(NH):
                            nc.tensor.matmul(
                                psum_b[tt][h][:],
                                silu2[:, tt * 128:(tt + 1) * 128],
                                wsn_sb[:, ic * OUT_F + h * 512: ic * OUT_F + h * 512 + 512],
                                start=(ic == 0), stop=(ic == NIC - 1 and not has_bias))
                if has_bias:
                    for tt in range(NTT):
                        for h in range(NH):
                            nc.tensor.matmul(
                                psum_b[tt][h][:], ones_sb[:],
                                bias_sb[:, h * 512:h * 512 + 512],
                                start=False, stop=True)
                base_sb = [[None] * NH for _ in range(NTT)]
                for tt in range(NTT):
                    for h in range(NH):
                        bt = bpool.tile([128, 512], BF16, tag="base")
                        nc.vector.tensor_copy(bt[:], psum_b[tt][h][:])
                        base_sb[tt][h] = bt

                # ---- phase B: wavelet basis matmuls ----
                psum_k = [[pp.tile([128, 512], F32, tag="ps", name=f"psk_{st}_{tt}_{h}")
                           for h in range(NH)] for tt in range(NTT)]
                for g in range(GRID):
                    for ic in range(NIC):
                        j = g * NIC + ic
                        sq = tpool.tile([128, ST], F32, tag="sq")
                        nc.scalar.activation(sq[:], xn_tiles[ic][:], AF.Square,
                                             scale=a_g[g], bias=const_col(b_g[g]))
                        e = tpool.tile([128, ST], BF16, tag="e")
                        nc.scalar.activation(e[:], sq[:], AF.Exp, scale=-0.5)
                        nb = tpool.tile([128, ST], BF16, tag="nb")
                        # -basis = (xe^2 - 1) * exp(-xe^2/2); ww is negated
                        nc.vector.scalar_tensor_tensor(
                            nb[:], sq[:], 1.0, e[:], OP.subtract, OP.mult)
                        first = (j == 0)
                        last = (j == GRID * NIC - 1)
                        for tt in range(NTT):
                            for h in range(NH):
                                nc.tensor.matmul(
                                    psum_k[tt][h][:],
                                    nb[:, tt * 128:(tt + 1) * 128],
                                    ww_sb[:, j * OUT_F + h * 512: j * OUT_F + h * 512 + 512],
                                    start=first, stop=last)

                # ---- epilogue: softshrink(kan) + base -> out ----
                for tt in range(NTT):
                    for h in range(NH):
                        r1 = epool.tile([128, 512], BF16, tag="r1")
                        nc.scalar.activation(r1[:], psum_k[tt][h][:], AF.Relu,
                                             bias=const_col(-thr), scale=1.0)
                        r2 = epool.tile([128, 512], BF16, tag="r2")
                        nc.scalar.activation(r2[:], psum_k[tt][h][:], AF.Relu,
                                             bias=const_col(-thr), scale=-1.0)
                        sh = epool.tile([128, 512], BF16, tag="sh")
                        nc.vector.tensor_sub(sh[:], r1[:], r2[:])
                        o_t = opool.tile([128, 512], F32, tag="o")
                        nc.vector.tensor_add(o_t[:], sh[:], base_sb[tt][h][:])
                        if out_scale != 1.0:
                            nc.vector.tensor_scalar_mul(o_t[:], o_t[:], out_scale)
                        nc.sync.dma_start(
                            out=out.ap()[s0 + tt * 128: s0 + (tt + 1) * 128,
                                         h * 512:h * 512 + 512],
                            in_=o_t[:])
    nc.compile()
    return nc


def kernel(x, base_w, base_b, u, translation, scale, wavelet_w, soft_threshold,
           output_scale):
    x = np.asarray(x, np.float32)
    base_w = np.asarray(base_w, np.float32)
    base_b = np.asarray(base_b, np.float32)
    u = np.asarray(u, np.float32)
    translation = np.asarray(translation, np.float32).reshape(-1)
    scale = np.asarray(scale, np.float32).reshape(-1)
    wavelet_w = np.asarray(wavelet_w, np.float32)
    thr = float(np.log1p(np.exp(np.float32(soft_threshold.reshape(-1)[0]))))
    out_scale = float(np.asarray(output_scale).reshape(-1)[0])

    # spectral norm (one power iteration, no-grad buffers) on host: O(IN*OUT)
    def l2n(v):
        return v / (np.linalg.norm(v) + np.float32(1e-12))
    v = l2n(base_w.T @ u)
    u2 = l2n(base_w @ v)
    sigma = u2 @ (base_w @ v)
    w_sn = base_w / sigma

    safe_s = np.maximum(np.abs(scale), np.float32(0.1))
    a_g = tuple(float(2.5 / safe_s[g]) for g in range(GRID))
    b_g = tuple(float(-translation[g] / safe_s[g]) for g in range(GRID))
    has_bias = bool(np.any(base_b != 0))

    key = (a_g, b_g, thr, out_scale, has_bias)
    if key not in _BUILD_CACHE:
        _BUILD_CACHE[key] = _build_nc(a_g, b_g, thr, out_scale, has_bias)
    nc = _BUILD_CACHE[key]

    # host-side weight prep (replicated across cores)
    bf16 = ml_dtypes.bfloat16
    # wsn[ic, i, o] = 0.5 * w_sn[o, ic*128+i]  (the 1/2 pairs with 2*silu)
    wsn_h = np.ascontiguousarray(
        (0.5 * w_sn.T).reshape(NIC, 128, OUT_F).astype(bf16))
    # ww[g*NIC+ic, i, o] = -wavelet_w[o, (ic*128+i)*GRID + g]
    w3 = wavelet_w.reshape(OUT_F, IN_F, GRID)          # [o, i, g]
    ww_h = np.ascontiguousarray(
        (-w3).transpose(2, 1, 0)                        # [g, i, o]
        .reshape(GRID, NIC, 128, OUT_F)
        .reshape(GRID * NIC, 128, OUT_F).astype(bf16))
    bias_h = np.ascontiguousarray(base_b.reshape(1, OUT_F).astype(bf16))

    x_flat = x.reshape(NTOK, IN_F)
    in_maps = []
    for c in range(N_CORES):
        xc = x_flat[c * TPC:(c + 1) * TPC]              # [TPC, IN_F]
        in_maps.append({
            "xT": np.ascontiguousarray(xc.T),           # [IN_F, TPC] f32
            "ww": ww_h,
            "wsn": wsn_h,
            "bias": bias_h,
        })

    res = run_bass_kernel_spmd(nc, in_maps, core_ids=list(range(N_CORES)))
    out = np.concatenate([res.results[c]["out"] for c in range(N_CORES)], axis=0)
    return out.reshape(BATCH, SEQ, OUT_F)
